# revision 8
# baseline (speedup 1.0000x reference)
"""Trainium2 Bass kernel for nn_DetJointBranch (detection joint branch).

Computation (per sample):
  conf branch: h = relu(BN(conv3x3(x, w1)))  [fused BN into weights]
               log_conf = conv1x1(h, w2) + b2
               conf = softmax(log_conf over full spatial map)
  sal branch:  log_sal = conv1x1(x4, ws)   (bs cancels in soft-NMS)
               sal = 3x3 soft-NMS(log_sal)
  score = sal * conf / max(conf)  == sal * exp(log_conf - max(log_conf))

Sharding: batch 4 x row-halves -> 8 cores. Core c: sample c//2, rows
(c%2)*128 .. +128. Cross-core softmax stats (max, sumexp) exchanged via a
tiny AllGather between half-pairs.

Matmuls run in fp16 (fp32 PSUM accumulation); everything else fp32.
"""

import sys

if '/opt/trn_rl_repo' not in sys.path:
    sys.path.insert(0, '/opt/trn_rl_repo')

import numpy as np

import concourse.bacc as bacc
import concourse.mybir as mybir
import concourse.tile as tile

N_CORES = 8
B, CIN, H, W = 4, 128, 256, 256
C4 = 64
COUT = 256
HH = 128          # rows per core (half image)
WP = 258          # padded row width (2 left zero-pad cols)
NEG = -1e30

F32 = mybir.dt.float32
F16 = mybir.dt.float16

_CACHE = {}


def _build():
    nc = bacc.Bacc("TRN2", target_bir_lowering=False, debug=False,
                   num_devices=N_CORES)

    xh = nc.dram_tensor("xh", [CIN, HH + 2, W], F32, kind="ExternalInput")
    x4h = nc.dram_tensor("x4h", [C4, HH + 4, W], F32, kind="ExternalInput")
    w1t = nc.dram_tensor("w1t", [CIN, 9, 2, 128], F16, kind="ExternalInput")
    b1h = nc.dram_tensor("b1h", [128, 2], F32, kind="ExternalInput")
    w2h = nc.dram_tensor("w2h", [128, 2], F16, kind="ExternalInput")
    b2s = nc.dram_tensor("b2s", [1], F32, kind="ExternalInput")
    wsh = nc.dram_tensor("wsh", [C4, 1], F16, kind="ExternalInput")
    maskh = nc.dram_tensor("maskh", [68, 4], F32, kind="ExternalInput")

    o_logconf = nc.dram_tensor("o_logconf", [HH, W], F32, kind="ExternalOutput")
    o_conf = nc.dram_tensor("o_conf", [HH, W], F32, kind="ExternalOutput")
    o_score = nc.dram_tensor("o_score", [HH, W], F32, kind="ExternalOutput")
    o_sal = nc.dram_tensor("o_sal", [HH, W], F32, kind="ExternalOutput")

    with tile.TileContext(nc) as tc:
        with (
            tc.tile_pool(name="const", bufs=1) as const,
            tc.tile_pool(name="persist", bufs=1) as persist,
            tc.tile_pool(name="xs", bufs=2) as xs_pool,
            tc.tile_pool(name="hp", bufs=8) as h_pool,
            tc.tile_pool(name="lc", bufs=2) as lc_pool,
            tc.tile_pool(name="x4s", bufs=2) as x4_pool,
            tc.tile_pool(name="lsl", bufs=2) as ls_pool,
            tc.tile_pool(name="nms", bufs=2) as nms_pool,
            tc.tile_pool(name="small", bufs=2) as small,
            tc.tile_pool(name="pconv", bufs=2, space="PSUM") as pc_pool,
            tc.tile_pool(name="pconf", bufs=1, space="PSUM") as plc_pool,
            tc.tile_pool(name="psal", bufs=1, space="PSUM") as pls_pool,
            tc.tile_pool(name="dram", bufs=1, space="DRAM") as dram,
        ):
            # ---- constants ----
            w1_sb = const.tile([CIN, 9, 2, 128], F16)
            nc.sync.dma_start(out=w1_sb[:], in_=w1t[:])
            b1_sb = const.tile([128, 2], F32)
            nc.sync.dma_start(out=b1_sb[:], in_=b1h[:])
            w2_sb = const.tile([128, 2], F16)
            nc.sync.dma_start(out=w2_sb[:], in_=w2h[:])
            ws_sb = const.tile([C4, 1], F16)
            nc.sync.dma_start(out=ws_sb[:], in_=wsh[:])
            b2_sb = const.tile([1, 1], F32)
            nc.sync.dma_start(out=b2_sb[0:1, :], in_=b2s[None, :])
            mask_sb = const.tile([68, 4], F32)
            nc.sync.dma_start(out=mask_sb[:], in_=maskh[:])

            # ---- persistent state ----
            conf_rows = persist.tile([HH, W], F32)     # log_conf (incl b2)
            sal_rows = persist.tile([HH, W], F32)
            # saliency row blocks (partition p = image-local row 64b-2+p / 64b-1+p)
            blks = [persist.tile([68, WP], F32, tag=f"blk{b}", name=f"blk{b}")
                    for b in (0, 1)]
            mids = [persist.tile([66, WP], F32, tag=f"mid{b}", name=f"mid{b}")
                    for b in (0, 1)]
            for t in blks + mids:
                nc.vector.memset(t[:, 0:1], NEG)
                nc.vector.memset(t[:, 257:258], NEG)

            # ls strip -> (target, dst_lo, src_lo, n) scatter plan
            # ls-index i (0..131) = image-local row i-2.
            def isect(lo, hi, t0):
                i0, i1 = max(lo, t0), min(hi, t0 + 12)
                return (i0, i1) if i1 > i0 else None

            lc_lin = None

            def conv_strip(s):
                nonlocal lc_lin
                x_sb = xs_pool.tile([CIN, 35, WP], F16, tag="xs")
                nc.vector.memset(x_sb[:, :, 0:2], 0.0)
                nc.vector.memset(x_sb[:, 34, :], 0.0)
                nc.gpsimd.dma_start(out=x_sb[:, 0:12, 2:258],
                                    in_=xh[:, 32 * s:32 * s + 12, :])
                nc.gpsimd.dma_start(out=x_sb[:, 12:34, 2:258],
                                    in_=xh[:, 32 * s + 12:32 * s + 34, :])
                xf = x_sb.rearrange("p a b -> p (a b)")
                # groups of 3 rows; tap-outer so all rows in a group share
                # one weight load (redundant LDWEIGHTS removed post-Tile)
                r_loc = 0
                for gn, gs in enumerate([3] * 10 + [2]):
                    rows = list(range(r_loc, r_loc + gs))
                    r_loc += gs
                    hs = [h_pool.tile([128, 2, WP], F16, tag="h",
                                      name=f"h{s}_{gn}_{j}")
                          for j in range(gs)]
                    for hf in range(2):
                        pc = pc_pool.tile([128, 3, 512], F32, tag="pc",
                                          name=f"pc{s}_{gn}_{hf}")
                        for tap in range(9):
                            dy, dx = divmod(tap, 3)
                            for j, r in enumerate(rows):
                                st = (r + dy) * WP + dx + 1
                                nc.tensor.matmul(
                                    pc[:, j, 0:WP],
                                    w1_sb[:, tap, hf, :],
                                    xf[:, st:st + WP],
                                    start=(tap == 0), stop=(tap == 8))
                        for j in range(gs):
                            nc.scalar.activation(
                                out=hs[j][:, hf, :], in_=pc[:, j, 0:WP],
                                func=mybir.ActivationFunctionType.Relu,
                                bias=b1_sb[:, hf:hf + 1], scale=1.0)
                    for j, r in enumerate(rows):
                        row = 32 * s + r
                        h_sb = hs[j]
                        plc = plc_pool.tile([1, WP], F32, tag="plc")
                        nc.tensor.matmul(plc[:], w2_sb[:, 0:1], h_sb[:, 0, :],
                                         start=True, stop=False)
                        nc.tensor.matmul(plc[:], w2_sb[:, 1:2], h_sb[:, 1, :],
                                         start=False, stop=True)
                        if row % 16 == 0:
                            lc_lin = lc_pool.tile([1, 16, WP], F32, tag="lc")
                        nc.scalar.activation(
                            out=lc_lin[0:1, row % 16, :], in_=plc[:],
                            func=mybir.ActivationFunctionType.Identity,
                            bias=b2_sb[0:1, 0:1], scale=1.0)
                        if row % 16 == 15:
                            g0 = row - 15
                            nc.sync.dma_start(out=conf_rows[g0:g0 + 16, :],
                                              in_=lc_lin[0:1, :, 0:W])

            def ls_strip(t):
                x4_sb = x4_pool.tile([C4, 12, W], F16, tag="x4")
                nc.gpsimd.dma_start(out=x4_sb[:], in_=x4h[:, 12 * t:12 * t + 12, :])
                x4f = x4_sb.rearrange("p a b -> p (a b)")
                ls_lin = ls_pool.tile([1, 12 * W], F32, tag="lsl")
                for k in range(6):
                    pls = pls_pool.tile([1, 512], F32, tag="pls")
                    nc.tensor.matmul(pls[:], ws_sb[:, 0:1],
                                     x4f[:, 512 * k:512 * k + 512],
                                     start=True, stop=True)
                    nc.vector.tensor_copy(ls_lin[0:1, 512 * k:512 * k + 512],
                                          pls[:])
                lsv = ls_lin.rearrange("p (a b) -> p a b", b=W)
                for tgt, lo, hi in (
                    (blks[0], 0, 68), (blks[1], 64, 132),
                    (mids[0], 1, 67), (mids[1], 65, 131),
                ):
                    iv = isect(lo, hi, 12 * t)
                    if iv:
                        i0, i1 = iv
                        nc.sync.dma_start(
                            out=tgt[i0 - lo:i1 - lo, 1:257],
                            in_=lsv[0:1, i0 - 12 * t:i1 - 12 * t, :])

            ls_sched = [(0, 1, 2), (3, 4, 5), (6, 7, 8), (9, 10)]
            for s in range(4):
                for t in ls_sched[s]:
                    ls_strip(t)
                conv_strip(s)

            # row-validity masks (NEG on out-of-image rows)
            for i, (tgt, n) in enumerate(
                    ((blks[0], 68), (blks[1], 68), (mids[0], 66), (mids[1], 66))):
                nc.vector.tensor_scalar_add(tgt[0:n, 1:257], tgt[0:n, 1:257],
                                            mask_sb[0:n, i:i + 1])

            # ---- soft NMS per 64-row block ----
            for bi in range(2):
                blk, mid = blks[bi], mids[bi]
                hmax = nms_pool.tile([68, W], F32, tag="hmax")
                nc.vector.tensor_max(hmax[:], blk[:, 0:256], blk[:, 1:257])
                nc.vector.tensor_max(hmax[:], hmax[:], blk[:, 2:258])
                hm1 = nms_pool.tile([66, W], F32, tag="hm1")
                nc.sync.dma_start(out=hm1[:], in_=hmax[1:67, :])
                hm2 = nms_pool.tile([66, W], F32, tag="hm2")
                nc.sync.dma_start(out=hm2[:], in_=hmax[2:68, :])
                vmax = nms_pool.tile([66, W], F32, tag="vmax")
                nc.vector.tensor_max(vmax[:], hmax[0:66, :], hm1[:])
                nc.vector.tensor_max(vmax[:], vmax[:], hm2[:])
                sub = nms_pool.tile([66, W], F32, tag="sub")
                nc.vector.tensor_sub(sub[:], mid[:, 1:257], vmax[:])
                e_blk = nms_pool.tile([66, WP], F32, tag="eblk")
                nc.vector.memset(e_blk[:, 0:1], 0.0)
                nc.vector.memset(e_blk[:, 257:258], 0.0)
                nc.scalar.activation(out=e_blk[:, 1:257], in_=sub[:],
                                     func=mybir.ActivationFunctionType.Exp)
                hsum = nms_pool.tile([66, W], F32, tag="hsum")
                nc.vector.tensor_add(hsum[:], e_blk[:, 0:256], e_blk[:, 1:257])
                nc.vector.tensor_add(hsum[:], hsum[:], e_blk[:, 2:258])
                hs1 = nms_pool.tile([64, W], F32, tag="hs1")
                nc.sync.dma_start(out=hs1[:], in_=hsum[1:65, :])
                hs2 = nms_pool.tile([64, W], F32, tag="hs2")
                nc.sync.dma_start(out=hs2[:], in_=hsum[2:66, :])
                den = nms_pool.tile([64, W], F32, tag="den")
                nc.vector.tensor_add(den[:], hsum[0:64, :], hs1[:])
                nc.vector.tensor_add(den[:], den[:], hs2[:])
                rec = nms_pool.tile([64, W], F32, tag="rec")
                nc.vector.reciprocal(rec[:], den[:])
                ectr = nms_pool.tile([64, W], F32, tag="ectr")
                nc.sync.dma_start(out=ectr[:], in_=e_blk[1:65, 1:257])
                salb = nms_pool.tile([64, W], F32, tag="salb")
                nc.vector.tensor_mul(salb[:], ectr[:], rec[:])
                nc.sync.dma_start(out=sal_rows[64 * bi:64 * bi + 64, :],
                                  in_=salb[:])

            # ---- softmax stats + pair exchange ----
            row_max = small.tile([128, 1], F32, tag="rmax")
            nc.vector.tensor_reduce(row_max[:], conf_rows[:],
                                    axis=mybir.AxisListType.X,
                                    op=mybir.AluOpType.max)
            e_tmp = persist.tile([HH, W], F32)
            row_sum = small.tile([128, 1], F32, tag="rsum")
            nc.scalar.activation(out=e_tmp[:], in_=conf_rows[:],
                                 func=mybir.ActivationFunctionType.Exp,
                                 accum_out=row_sum[:])
            rm_lin = small.tile([1, 128], F32, tag="rml")
            nc.sync.dma_start(out=rm_lin[0:1, :], in_=row_max[:])
            rs_lin = small.tile([1, 128], F32, tag="rsl")
            nc.sync.dma_start(out=rs_lin[0:1, :], in_=row_sum[:])
            stat_sb = small.tile([1, 64], F32, tag="stat")
            nc.vector.memset(stat_sb[:], 0.0)
            nc.vector.tensor_reduce(stat_sb[0:1, 0:1], rm_lin[0:1, :],
                                    axis=mybir.AxisListType.X,
                                    op=mybir.AluOpType.max)
            nc.vector.tensor_reduce(stat_sb[0:1, 1:2], rs_lin[0:1, :],
                                    axis=mybir.AxisListType.X,
                                    op=mybir.AluOpType.add)
            cstat_in = dram.tile([1, 64], F32)
            cstat_out = dram.tile([1, 128], F32)
            nc.sync.dma_start(out=cstat_in[:], in_=stat_sb[:])
            nc.gpsimd.collective_compute(
                "AllGather", mybir.AluOpType.bypass,
                replica_groups=[[0, 1], [2, 3], [4, 5], [6, 7]],
                ins=[cstat_in.opt()], outs=[cstat_out.opt()])
            cg = small.tile([1, 128], F32, tag="cg")
            nc.sync.dma_start(out=cg[:], in_=cstat_out[:])
            m_g = small.tile([1, 1], F32, tag="mg")
            nc.vector.tensor_max(m_g[:], cg[0:1, 0:1], cg[0:1, 64:65])
            s_g = small.tile([1, 1], F32, tag="sg")
            nc.vector.tensor_add(s_g[:], cg[0:1, 1:2], cg[0:1, 65:66])
            rec_s = small.tile([1, 1], F32, tag="recs")
            nc.vector.reciprocal(rec_s[:], s_g[:])
            inv_em = small.tile([1, 1], F32, tag="invem")
            nc.scalar.activation(out=inv_em[:], in_=m_g[:],
                                 func=mybir.ActivationFunctionType.Exp,
                                 scale=-1.0)
            rs_b = small.tile([128, 1], F32, tag="rsb")
            nc.gpsimd.partition_broadcast(rs_b[:], rec_s[0:1, :])
            em_b = small.tile([128, 1], F32, tag="emb")
            nc.gpsimd.partition_broadcast(em_b[:], inv_em[0:1, :])

            conf_sb = persist.tile([HH, W], F32)
            nc.vector.tensor_scalar_mul(conf_sb[:], e_tmp[:], rs_b[:, 0:1])
            escore = persist.tile([HH, W], F32)
            nc.vector.tensor_scalar_mul(escore[:], e_tmp[:], em_b[:, 0:1])
            score_sb = persist.tile([HH, W], F32)
            nc.vector.tensor_mul(score_sb[:], escore[:], sal_rows[:])

            nc.sync.dma_start(out=o_logconf[:], in_=conf_rows[:])
            nc.sync.dma_start(out=o_conf[:], in_=conf_sb[:])
            nc.sync.dma_start(out=o_score[:], in_=score_sb[:])
            nc.sync.dma_start(out=o_sal[:], in_=sal_rows[:])

    _dedup_ldweights(nc)
    nc.compile()
    return nc


def _ldw_sig(ins):
    arg = ins.ins[0]
    bap = getattr(arg, "bass_ap", None)
    if bap is None:
        return None
    t = bap.tensor
    name = getattr(t, "name", None)
    if name is None or not any(k in name for k in ("w1_sb", "w2_sb", "ws_sb")):
        return None  # only const, write-once weight tensors are safe
    return (name, bap.offset, tuple(map(tuple, bap.ap)))


def _dedup_ldweights(nc):
    """Remove LDWEIGHTS that reload the weights already resident in the PE
    array (same const tensor slice as the previous load in the PE stream)."""
    import concourse.bass as bass
    pe = mybir.EngineType.PE
    removed = 0
    for blk in nc.main_func.blocks:
        last_sig = None
        pending = None
        keep = []
        for ins in blk.instructions:
            eng = getattr(ins, "engine", None)
            if isinstance(ins, mybir.InstLdweights):
                sig = _ldw_sig(ins)
                if sig is not None and sig == last_sig:
                    si = ins.sync_info
                    if si is not None and (si.on_wait or si.on_update):
                        assert pending is None
                        pending = si
                    removed += 1
                    continue
                last_sig = sig
            elif isinstance(ins, mybir.InstMatmult):
                if pending is not None:
                    si = ins.sync_info
                    if si is None:
                        ins.sync_info = pending
                    else:
                        si.on_wait = list(pending.on_wait) + list(si.on_wait)
                        si.on_update = (list(pending.on_update)
                                        + list(si.on_update))
                    pending = None
            elif eng == pe and not isinstance(
                    ins, (mybir.InstEventSemaphore, mybir.InstDrain)):
                last_sig = None  # unknown PE instruction: be conservative
            keep.append(ins)
        assert pending is None
        blk.instructions[:] = keep
    return removed


def get_nc():
    if "nc" not in _CACHE:
        _CACHE["nc"] = _build()
    return _CACHE["nc"]


BN_EPS = 1e-5


def make_in_maps(x, x4, w1, b1, bn_gamma, bn_beta, bn_mean, bn_var, w2, b2,
                 ws, bs, soft_nms_kernel_size):
    assert int(soft_nms_kernel_size) == 3
    x = np.asarray(x, np.float32)
    x4 = np.asarray(x4, np.float32)
    inv = np.asarray(bn_gamma, np.float32) / np.sqrt(
        np.asarray(bn_var, np.float32) + BN_EPS)
    w1f = np.asarray(w1, np.float32) * inv[:, None, None, None]
    b1f = (np.asarray(b1, np.float32) - np.asarray(bn_mean, np.float32)) * inv \
        + np.asarray(bn_beta, np.float32)
    # [oc, ic, dy, dx] -> [ic, dy*3+dx, half, oc%128]
    w1t = np.ascontiguousarray(
        w1f.transpose(1, 2, 3, 0).reshape(CIN, 9, 2, 128).astype(np.float16))
    b1h = np.ascontiguousarray(b1f.reshape(2, 128).T.astype(np.float32))
    w2h = np.ascontiguousarray(
        np.asarray(w2, np.float32)[0, :, 0, 0].reshape(2, 128).T.astype(np.float16))
    b2s = np.asarray(b2, np.float32).reshape(1)
    wsh = np.ascontiguousarray(
        np.asarray(ws, np.float32)[0, :, 0, 0].reshape(C4, 1).astype(np.float16))

    xp = np.pad(x, ((0, 0), (0, 0), (1, 1), (0, 0)))
    x4p = np.pad(x4, ((0, 0), (0, 0), (2, 2), (0, 0)))

    in_maps = []
    for c in range(N_CORES):
        b, half = divmod(c, 2)
        g0 = HH * half
        mask = np.zeros((68, 4), np.float32)
        for i, (lo, n) in enumerate(((-2, 68), (62, 68), (-1, 66), (63, 66))):
            for p in range(n):
                gl = g0 + lo + p  # global image row of partition p
                if gl < 0 or gl >= H:
                    mask[p, i] = NEG
        in_maps.append({
            "xh": np.ascontiguousarray(xp[b, :, g0:g0 + HH + 2, :]),
            "x4h": np.ascontiguousarray(x4p[b, :, g0:g0 + HH + 4, :]),
            "w1t": w1t, "b1h": b1h, "w2h": w2h, "b2s": b2s, "wsh": wsh,
            "maskh": mask,
        })
    return in_maps


def run(in_maps, trace=False, **kw):
    from concourse.bass_utils import run_bass_kernel_spmd
    return run_bass_kernel_spmd(get_nc(), in_maps, list(range(N_CORES)),
                                trace=trace, **kw)


def kernel(**inputs):
    in_maps = make_in_maps(**inputs)
    res = run(in_maps)
    score = np.zeros((B, 1, H, W), np.float32)
    conf = np.zeros((B, 1, H, W), np.float32)
    logc = np.zeros((B, 1, H, W), np.float32)
    sal = np.zeros((B, 1, H, W), np.float32)
    for c in range(N_CORES):
        b, half = divmod(c, 2)
        g0 = HH * half
        r = res.results[c]
        score[b, 0, g0:g0 + HH] = r["o_score"]
        conf[b, 0, g0:g0 + HH] = r["o_conf"]
        logc[b, 0, g0:g0 + HH] = r["o_logconf"]
        sal[b, 0, g0:g0 + HH] = r["o_sal"]
    return (score, conf, logc, sal)


# revision 10
# speedup vs baseline: 1.0489x; 1.0489x over previous
"""Trainium2 Bass kernel for nn_DetJointBranch (detection joint branch).

Computation (per sample):
  conf branch: h = relu(BN(conv3x3(x, w1)))  [fused BN into weights]
               log_conf = conv1x1(h, w2) + b2
               conf = softmax(log_conf over full spatial map)
  sal branch:  log_sal = conv1x1(x4, ws)   (bs cancels in soft-NMS)
               sal = 3x3 soft-NMS(log_sal)
  score = sal * conf / max(conf)  == sal * exp(log_conf - max(log_conf))

Sharding: batch 4 x row-halves -> 8 cores. Core c: sample c//2, rows
(c%2)*128 .. +128. Cross-core softmax stats (max, sumexp) exchanged via a
tiny AllGather between half-pairs.

Matmuls run in fp16 (fp32 PSUM accumulation); everything else fp32.
"""

import sys

if '/opt/trn_rl_repo' not in sys.path:
    sys.path.insert(0, '/opt/trn_rl_repo')

import numpy as np

import concourse.bacc as bacc
import concourse.mybir as mybir
import concourse.tile as tile

N_CORES = 8
B, CIN, H, W = 4, 128, 256, 256
C4 = 64
COUT = 256
HH = 128          # rows per core (half image)
WP = 258          # padded row width (2 left zero-pad cols)
NEG = -1e30

F32 = mybir.dt.float32
F16 = mybir.dt.float16

_CACHE = {}


def _build():
    nc = bacc.Bacc("TRN2", target_bir_lowering=False, debug=False,
                   num_devices=N_CORES)

    xh = nc.dram_tensor("xh", [CIN, HH + 2, W], F32, kind="ExternalInput")
    x4h = nc.dram_tensor("x4h", [C4, HH + 4, W], F32, kind="ExternalInput")
    w1t = nc.dram_tensor("w1t", [CIN, 9, 2, 128], F16, kind="ExternalInput")
    b1h = nc.dram_tensor("b1h", [128, 2], F32, kind="ExternalInput")
    w2h = nc.dram_tensor("w2h", [128, 2], F16, kind="ExternalInput")
    b2s = nc.dram_tensor("b2s", [1], F32, kind="ExternalInput")
    wsh = nc.dram_tensor("wsh", [C4, 1], F16, kind="ExternalInput")
    maskh = nc.dram_tensor("maskh", [68, 4], F32, kind="ExternalInput")

    o_logconf = nc.dram_tensor("o_logconf", [HH, W], F32, kind="ExternalOutput")
    o_conf = nc.dram_tensor("o_conf", [HH, W], F32, kind="ExternalOutput")
    o_score = nc.dram_tensor("o_score", [HH, W], F32, kind="ExternalOutput")
    o_sal = nc.dram_tensor("o_sal", [HH, W], F32, kind="ExternalOutput")

    with tile.TileContext(nc) as tc:
        with (
            tc.tile_pool(name="const", bufs=1) as const,
            tc.tile_pool(name="persist", bufs=1) as persist,
            tc.tile_pool(name="xs", bufs=3) as xs_pool,
            tc.tile_pool(name="hp", bufs=8) as h_pool,
            tc.tile_pool(name="lc", bufs=2) as lc_pool,
            tc.tile_pool(name="x4s", bufs=4) as x4_pool,
            tc.tile_pool(name="lsl", bufs=2) as ls_pool,
            tc.tile_pool(name="nms", bufs=2) as nms_pool,
            tc.tile_pool(name="small", bufs=2) as small,
            tc.tile_pool(name="pconv", bufs=2, space="PSUM") as pc_pool,
            tc.tile_pool(name="pconf", bufs=1, space="PSUM") as plc_pool,
            tc.tile_pool(name="psal", bufs=1, space="PSUM") as pls_pool,
            tc.tile_pool(name="dram", bufs=1, space="DRAM") as dram,
        ):
            # ---- constants ----
            w1_sb = const.tile([CIN, 9, 2, 128], F16)
            nc.sync.dma_start(out=w1_sb[:], in_=w1t[:])
            b1_sb = const.tile([128, 2], F32)
            nc.sync.dma_start(out=b1_sb[:], in_=b1h[:])
            w2_sb = const.tile([128, 2], F16)
            nc.sync.dma_start(out=w2_sb[:], in_=w2h[:])
            ws_sb = const.tile([C4, 1], F16)
            nc.sync.dma_start(out=ws_sb[:], in_=wsh[:])
            b2_sb = const.tile([1, 1], F32)
            nc.sync.dma_start(out=b2_sb[0:1, :], in_=b2s[None, :])
            mask_sb = const.tile([68, 4], F32)
            nc.sync.dma_start(out=mask_sb[:], in_=maskh[:])

            # ---- persistent state ----
            conf_rows = persist.tile([HH, W], F32)     # log_conf (incl b2)
            e_tmp = persist.tile([HH, W], F32)         # exp(log_conf)
            row_max = persist.tile([128, 1], F32)
            row_sum = persist.tile([128, 1], F32)
            sal_rows = persist.tile([HH, W], F32)
            # saliency row blocks (partition p = image-local row 64b-2+p / 64b-1+p)
            blks = [persist.tile([68, WP], F32, tag=f"blk{b}", name=f"blk{b}")
                    for b in (0, 1)]
            mids = [persist.tile([66, WP], F32, tag=f"mid{b}", name=f"mid{b}")
                    for b in (0, 1)]
            for t in blks + mids:
                nc.vector.memset(t[:, 0:1], NEG)
                nc.vector.memset(t[:, 257:258], NEG)

            # ls strip -> (target, dst_lo, src_lo, n) scatter plan
            # ls-index i (0..131) = image-local row i-2.
            def isect(lo, hi, t0):
                i0, i1 = max(lo, t0), min(hi, t0 + 12)
                return (i0, i1) if i1 > i0 else None

            lc_lin = None

            def conv_strip(s):
                nonlocal lc_lin
                x_sb = xs_pool.tile([CIN, 35, WP], F16, tag="xs")
                nc.vector.memset(x_sb[:, :, 0:2], 0.0)
                nc.vector.memset(x_sb[:, 34, :], 0.0)
                nc.gpsimd.dma_start(out=x_sb[:, 0:12, 2:258],
                                    in_=xh[:, 32 * s:32 * s + 12, :])
                nc.gpsimd.dma_start(out=x_sb[:, 12:34, 2:258],
                                    in_=xh[:, 32 * s + 12:32 * s + 34, :])
                xf = x_sb.rearrange("p a b -> p (a b)")
                # groups of 3 rows; tap-outer so all rows in a group share
                # one weight load (redundant LDWEIGHTS removed post-Tile)
                r_loc = 0
                for gn, gs in enumerate([3] * 10 + [2]):
                    rows = list(range(r_loc, r_loc + gs))
                    r_loc += gs
                    hs = [h_pool.tile([128, 2, WP], F16, tag="h",
                                      name=f"h{s}_{gn}_{j}")
                          for j in range(gs)]
                    for hf in range(2):
                        pc = pc_pool.tile([128, 3, 512], F32, tag="pc",
                                          name=f"pc{s}_{gn}_{hf}")
                        for tap in range(9):
                            dy, dx = divmod(tap, 3)
                            for j, r in enumerate(rows):
                                st = (r + dy) * WP + dx + 1
                                nc.tensor.matmul(
                                    pc[:, j, 0:WP],
                                    w1_sb[:, tap, hf, :],
                                    xf[:, st:st + WP],
                                    start=(tap == 0), stop=(tap == 8))
                        for j in range(gs):
                            nc.scalar.activation(
                                out=hs[j][:, hf, :], in_=pc[:, j, 0:WP],
                                func=mybir.ActivationFunctionType.Relu,
                                bias=b1_sb[:, hf:hf + 1], scale=1.0)
                    for j, r in enumerate(rows):
                        row = 32 * s + r
                        h_sb = hs[j]
                        plc = plc_pool.tile([1, WP], F32, tag="plc")
                        nc.tensor.matmul(plc[:], w2_sb[:, 0:1], h_sb[:, 0, :],
                                         start=True, stop=False)
                        nc.tensor.matmul(plc[:], w2_sb[:, 1:2], h_sb[:, 1, :],
                                         start=False, stop=True)
                        if row % 16 == 0:
                            lc_lin = lc_pool.tile([1, 16, WP], F32, tag="lc")
                        nc.scalar.activation(
                            out=lc_lin[0:1, row % 16, :], in_=plc[:],
                            func=mybir.ActivationFunctionType.Identity,
                            bias=b2_sb[0:1, 0:1], scale=1.0)
                        if row % 16 == 15:
                            g0 = row - 15
                            nc.sync.dma_start(out=conf_rows[g0:g0 + 16, :],
                                              in_=lc_lin[0:1, :, 0:W])
                            nc.sync.dma_start(out=o_logconf[g0:g0 + 16, :],
                                              in_=lc_lin[0:1, :, 0:W])
                        if row % 32 == 31:
                            q0 = row - 31
                            nc.vector.tensor_reduce(
                                row_max[q0:q0 + 32, :],
                                conf_rows[q0:q0 + 32, :],
                                axis=mybir.AxisListType.X,
                                op=mybir.AluOpType.max)
                            nc.scalar.activation(
                                out=e_tmp[q0:q0 + 32, :],
                                in_=conf_rows[q0:q0 + 32, :],
                                func=mybir.ActivationFunctionType.Exp,
                                accum_out=row_sum[q0:q0 + 32, :])

            def ls_strip(t):
                x4_sb = x4_pool.tile([C4, 12, W], F16, tag="x4")
                nc.gpsimd.dma_start(out=x4_sb[:], in_=x4h[:, 12 * t:12 * t + 12, :])
                x4f = x4_sb.rearrange("p a b -> p (a b)")
                ls_lin = ls_pool.tile([1, 12 * W], F32, tag="lsl")
                for k in range(6):
                    pls = pls_pool.tile([1, 512], F32, tag="pls")
                    nc.tensor.matmul(pls[:], ws_sb[:, 0:1],
                                     x4f[:, 512 * k:512 * k + 512],
                                     start=True, stop=True)
                    nc.vector.tensor_copy(ls_lin[0:1, 512 * k:512 * k + 512],
                                          pls[:])
                lsv = ls_lin.rearrange("p (a b) -> p a b", b=W)
                for tgt, lo, hi in (
                    (blks[0], 0, 68), (blks[1], 64, 132),
                    (mids[0], 1, 67), (mids[1], 65, 131),
                ):
                    iv = isect(lo, hi, 12 * t)
                    if iv:
                        i0, i1 = iv
                        nc.sync.dma_start(
                            out=tgt[i0 - lo:i1 - lo, 1:257],
                            in_=lsv[0:1, i0 - 12 * t:i1 - 12 * t, :])

            ls_sched = [(), (0, 1, 2, 3), (4, 5, 6, 7), (8, 9, 10)]
            for s in range(4):
                for t in ls_sched[s]:
                    ls_strip(t)
                conv_strip(s)

            # row-validity masks (NEG on out-of-image rows)
            for i, (tgt, n) in enumerate(
                    ((blks[0], 68), (blks[1], 68), (mids[0], 66), (mids[1], 66))):
                nc.vector.tensor_scalar_add(tgt[0:n, 1:257], tgt[0:n, 1:257],
                                            mask_sb[0:n, i:i + 1])

            # ---- soft NMS per 64-row block ----
            for bi in range(2):
                blk, mid = blks[bi], mids[bi]
                hmax = nms_pool.tile([68, W], F32, tag="hmax")
                nc.vector.tensor_max(hmax[:], blk[:, 0:256], blk[:, 1:257])
                nc.vector.tensor_max(hmax[:], hmax[:], blk[:, 2:258])
                hm1 = nms_pool.tile([66, W], F32, tag="hm1")
                nc.sync.dma_start(out=hm1[:], in_=hmax[1:67, :])
                hm2 = nms_pool.tile([66, W], F32, tag="hm2")
                nc.sync.dma_start(out=hm2[:], in_=hmax[2:68, :])
                vmax = nms_pool.tile([66, W], F32, tag="vmax")
                nc.vector.tensor_max(vmax[:], hmax[0:66, :], hm1[:])
                nc.vector.tensor_max(vmax[:], vmax[:], hm2[:])
                sub = nms_pool.tile([66, W], F32, tag="sub")
                nc.vector.tensor_sub(sub[:], mid[:, 1:257], vmax[:])
                e_blk = nms_pool.tile([66, WP], F32, tag="eblk")
                nc.vector.memset(e_blk[:, 0:1], 0.0)
                nc.vector.memset(e_blk[:, 257:258], 0.0)
                nc.scalar.activation(out=e_blk[:, 1:257], in_=sub[:],
                                     func=mybir.ActivationFunctionType.Exp)
                hsum = nms_pool.tile([66, W], F32, tag="hsum")
                nc.vector.tensor_add(hsum[:], e_blk[:, 0:256], e_blk[:, 1:257])
                nc.vector.tensor_add(hsum[:], hsum[:], e_blk[:, 2:258])
                hs1 = nms_pool.tile([64, W], F32, tag="hs1")
                nc.sync.dma_start(out=hs1[:], in_=hsum[1:65, :])
                hs2 = nms_pool.tile([64, W], F32, tag="hs2")
                nc.sync.dma_start(out=hs2[:], in_=hsum[2:66, :])
                den = nms_pool.tile([64, W], F32, tag="den")
                nc.vector.tensor_add(den[:], hsum[0:64, :], hs1[:])
                nc.vector.tensor_add(den[:], den[:], hs2[:])
                rec = nms_pool.tile([64, W], F32, tag="rec")
                nc.vector.reciprocal(rec[:], den[:])
                ectr = nms_pool.tile([64, W], F32, tag="ectr")
                nc.sync.dma_start(out=ectr[:], in_=e_blk[1:65, 1:257])
                salb = nms_pool.tile([64, W], F32, tag="salb")
                nc.vector.tensor_mul(salb[:], ectr[:], rec[:])
                nc.sync.dma_start(out=sal_rows[64 * bi:64 * bi + 64, :],
                                  in_=salb[:])

            # ---- softmax stats + pair exchange ----
            rm_lin = small.tile([1, 128], F32, tag="rml")
            nc.sync.dma_start(out=rm_lin[0:1, :], in_=row_max[:])
            rs_lin = small.tile([1, 128], F32, tag="rsl")
            nc.sync.dma_start(out=rs_lin[0:1, :], in_=row_sum[:])
            stat_sb = small.tile([1, 64], F32, tag="stat")
            nc.vector.memset(stat_sb[:], 0.0)
            nc.vector.tensor_reduce(stat_sb[0:1, 0:1], rm_lin[0:1, :],
                                    axis=mybir.AxisListType.X,
                                    op=mybir.AluOpType.max)
            nc.vector.tensor_reduce(stat_sb[0:1, 1:2], rs_lin[0:1, :],
                                    axis=mybir.AxisListType.X,
                                    op=mybir.AluOpType.add)
            cstat_in = dram.tile([1, 64], F32)
            cstat_out = dram.tile([1, 128], F32)
            nc.sync.dma_start(out=cstat_in[:], in_=stat_sb[:])
            nc.gpsimd.collective_compute(
                "AllGather", mybir.AluOpType.bypass,
                replica_groups=[[0, 1], [2, 3], [4, 5], [6, 7]],
                ins=[cstat_in.opt()], outs=[cstat_out.opt()])
            cg = small.tile([1, 128], F32, tag="cg")
            nc.sync.dma_start(out=cg[:], in_=cstat_out[:])
            m_g = small.tile([1, 1], F32, tag="mg")
            nc.vector.tensor_max(m_g[:], cg[0:1, 0:1], cg[0:1, 64:65])
            s_g = small.tile([1, 1], F32, tag="sg")
            nc.vector.tensor_add(s_g[:], cg[0:1, 1:2], cg[0:1, 65:66])
            rec_s = small.tile([1, 1], F32, tag="recs")
            nc.vector.reciprocal(rec_s[:], s_g[:])
            inv_em = small.tile([1, 1], F32, tag="invem")
            nc.scalar.activation(out=inv_em[:], in_=m_g[:],
                                 func=mybir.ActivationFunctionType.Exp,
                                 scale=-1.0)
            rs_b = small.tile([128, 1], F32, tag="rsb")
            nc.gpsimd.partition_broadcast(rs_b[:], rec_s[0:1, :])
            em_b = small.tile([128, 1], F32, tag="emb")
            nc.gpsimd.partition_broadcast(em_b[:], inv_em[0:1, :])

            conf_sb = persist.tile([HH, W], F32)
            nc.vector.tensor_scalar_mul(conf_sb[:], e_tmp[:], rs_b[:, 0:1])
            escore = persist.tile([HH, W], F32)
            nc.vector.tensor_scalar_mul(escore[:], e_tmp[:], em_b[:, 0:1])
            score_sb = persist.tile([HH, W], F32)
            nc.vector.tensor_mul(score_sb[:], escore[:], sal_rows[:])

            nc.sync.dma_start(out=o_conf[:], in_=conf_sb[:])
            nc.sync.dma_start(out=o_score[:], in_=score_sb[:])
            nc.sync.dma_start(out=o_sal[:], in_=sal_rows[:])

    _dedup_ldweights(nc)
    nc.compile()
    return nc


def _ldw_sig(ins):
    arg = ins.ins[0]
    bap = getattr(arg, "bass_ap", None)
    if bap is None:
        return None
    t = bap.tensor
    name = getattr(t, "name", None)
    if name is None or not any(k in name for k in ("w1_sb", "w2_sb", "ws_sb")):
        return None  # only const, write-once weight tensors are safe
    return (name, bap.offset, tuple(map(tuple, bap.ap)))


def _dedup_ldweights(nc):
    """Remove LDWEIGHTS that reload the weights already resident in the PE
    array (same const tensor slice as the previous load in the PE stream)."""
    import concourse.bass as bass
    pe = mybir.EngineType.PE
    removed = 0
    for blk in nc.main_func.blocks:
        last_sig = None
        pending = None
        keep = []
        for ins in blk.instructions:
            eng = getattr(ins, "engine", None)
            if isinstance(ins, mybir.InstLdweights):
                sig = _ldw_sig(ins)
                if sig is not None and sig == last_sig:
                    si = ins.sync_info
                    if si is not None and (si.on_wait or si.on_update):
                        assert pending is None
                        pending = si
                    removed += 1
                    continue
                last_sig = sig
            elif isinstance(ins, mybir.InstMatmult):
                if pending is not None:
                    si = ins.sync_info
                    if si is None:
                        ins.sync_info = pending
                    else:
                        si.on_wait = list(pending.on_wait) + list(si.on_wait)
                        si.on_update = (list(pending.on_update)
                                        + list(si.on_update))
                    pending = None
            elif eng == pe and not isinstance(
                    ins, (mybir.InstEventSemaphore, mybir.InstDrain)):
                last_sig = None  # unknown PE instruction: be conservative
            keep.append(ins)
        assert pending is None
        blk.instructions[:] = keep
    return removed


def get_nc():
    if "nc" not in _CACHE:
        _CACHE["nc"] = _build()
    return _CACHE["nc"]


BN_EPS = 1e-5


def make_in_maps(x, x4, w1, b1, bn_gamma, bn_beta, bn_mean, bn_var, w2, b2,
                 ws, bs, soft_nms_kernel_size):
    assert int(soft_nms_kernel_size) == 3
    x = np.asarray(x, np.float32)
    x4 = np.asarray(x4, np.float32)
    inv = np.asarray(bn_gamma, np.float32) / np.sqrt(
        np.asarray(bn_var, np.float32) + BN_EPS)
    w1f = np.asarray(w1, np.float32) * inv[:, None, None, None]
    b1f = (np.asarray(b1, np.float32) - np.asarray(bn_mean, np.float32)) * inv \
        + np.asarray(bn_beta, np.float32)
    # [oc, ic, dy, dx] -> [ic, dy*3+dx, half, oc%128]
    w1t = np.ascontiguousarray(
        w1f.transpose(1, 2, 3, 0).reshape(CIN, 9, 2, 128).astype(np.float16))
    b1h = np.ascontiguousarray(b1f.reshape(2, 128).T.astype(np.float32))
    w2h = np.ascontiguousarray(
        np.asarray(w2, np.float32)[0, :, 0, 0].reshape(2, 128).T.astype(np.float16))
    b2s = np.asarray(b2, np.float32).reshape(1)
    wsh = np.ascontiguousarray(
        np.asarray(ws, np.float32)[0, :, 0, 0].reshape(C4, 1).astype(np.float16))

    xp = np.pad(x, ((0, 0), (0, 0), (1, 1), (0, 0)))
    x4p = np.pad(x4, ((0, 0), (0, 0), (2, 2), (0, 0)))

    in_maps = []
    for c in range(N_CORES):
        b, half = divmod(c, 2)
        g0 = HH * half
        mask = np.zeros((68, 4), np.float32)
        for i, (lo, n) in enumerate(((-2, 68), (62, 68), (-1, 66), (63, 66))):
            for p in range(n):
                gl = g0 + lo + p  # global image row of partition p
                if gl < 0 or gl >= H:
                    mask[p, i] = NEG
        in_maps.append({
            "xh": np.ascontiguousarray(xp[b, :, g0:g0 + HH + 2, :]),
            "x4h": np.ascontiguousarray(x4p[b, :, g0:g0 + HH + 4, :]),
            "w1t": w1t, "b1h": b1h, "w2h": w2h, "b2s": b2s, "wsh": wsh,
            "maskh": mask,
        })
    return in_maps


def run(in_maps, trace=False, **kw):
    from concourse.bass_utils import run_bass_kernel_spmd
    return run_bass_kernel_spmd(get_nc(), in_maps, list(range(N_CORES)),
                                trace=trace, **kw)


def kernel(**inputs):
    in_maps = make_in_maps(**inputs)
    res = run(in_maps)
    score = np.zeros((B, 1, H, W), np.float32)
    conf = np.zeros((B, 1, H, W), np.float32)
    logc = np.zeros((B, 1, H, W), np.float32)
    sal = np.zeros((B, 1, H, W), np.float32)
    for c in range(N_CORES):
        b, half = divmod(c, 2)
        g0 = HH * half
        r = res.results[c]
        score[b, 0, g0:g0 + HH] = r["o_score"]
        conf[b, 0, g0:g0 + HH] = r["o_conf"]
        logc[b, 0, g0:g0 + HH] = r["o_logconf"]
        sal[b, 0, g0:g0 + HH] = r["o_sal"]
    return (score, conf, logc, sal)
